# revision 1
# baseline (speedup 1.0000x reference)
"""MultiHeadAttention (B=2, S=2048, D=1024, H=16) on 8 TRN2 NeuronCores.

Sharding: core c -> batch b = c//4, head-group g = c%4 (4 heads = 256 channels).
Each core computes its 4 heads' attention for its batch plus the partial
out-projection (out_w columns for its channel group); host sums the 4 partials
per batch and adds out_b.

Device kernel (per core, all matmuls fp32r):
  phase A: QT,KT [256,2048] (j-major) and V_aug [2048, 4x65] (ones column per
           head appended -> softmax denominators fall out of the AV matmul).
  phase B: per 512-query tile x 128-key chunk: logitsT = K Q^T via row-packed
           pairs (K=64 each), exp on ACT (no max subtraction; |logit| <~ 50),
           AV accumulation with M=65; then reciprocal + ones-matmul broadcast
           + normalize.
  phase C: out-projection partial [2048, 1024] -> DRAM.
"""

import os
import sys

import numpy as np

for _p in ("/opt/trn_rl_repo",):
    if os.path.isdir(_p) and _p not in sys.path:
        sys.path.insert(0, _p)

from contextlib import ExitStack

import concourse.bass as bass
import concourse.tile as tile
from concourse import bacc, mybir
from concourse._compat import with_exitstack
from concourse.bass_utils import run_bass_kernel_spmd

B, S, D = 2, 2048, 1024
H = 16
HD = 64
NCORES = 8
JG = 256          # channels per core (4 heads)
DC = D // 128     # 8 contraction chunks
QT_TILES = 4      # 4 x 512 query tiles
KC = S // 128     # 16 key chunks
VW = 65           # V columns per head incl. ones column
FP32 = mybir.dt.float32
FP32R = mybir.dt.float32r
EXP = mybir.ActivationFunctionType.Exp


@with_exitstack
def mha_core_kernel(ctx: ExitStack, tc: tile.TileContext,
                    out, xT, wqT, wkT, wvT, bq, bk, bv, owT):
    nc = tc.nc
    # fp32r SBUF tiles hold full fp32 bit patterns; only the PE rounds.
    ctx.enter_context(nc.allow_low_precision("fp32r tiles carry fp32 bits"))

    persist = ctx.enter_context(tc.tile_pool(name="persist", bufs=1))
    QT_sb = persist.tile((128, 2 * S), FP32R)
    KT_sb = persist.tile((128, 2 * S), FP32R)
    Vaug_sb = persist.tile((128, KC * 4 * VW), FP32R)
    attn_outT_sb = persist.tile((128, 2 * S), FP32R)
    owT_sb = persist.tile((128, 2 * D), FP32R)
    ones2_sb = persist.tile((2, 128), FP32R)
    ones1_sb = persist.tile((1, 128), FP32R)

    # ---------------- phase A: QKV projections ----------------
    with tc.tile_pool(name="pA", bufs=1) as pA, \
         tc.tile_pool(name="psA", bufs=1, space="PSUM") as psA:
        xT_sb = pA.tile((128, DC * S), FP32R)
        wqT_sb = pA.tile((128, DC * JG), FP32R)
        wkT_sb = pA.tile((128, DC * JG), FP32R)
        wvT_sb = pA.tile((128, DC * JG), FP32R)
        bq_sb = pA.tile((128, 2), FP32)
        bk_sb = pA.tile((128, 2), FP32)
        bv_bc = pA.tile((128, JG), FP32)
        ones_f32 = pA.tile((128, 64), FP32)
        ones2_f32 = pA.tile((2, 128), FP32)

        # Memset can't emit fp32r (ISA); stage fp32 ones and DVE-copy them
        # into the fp32r tiles (the per-head ones columns of Vaug + the
        # 2-row block-ones used for the packed denominator broadcast).
        nc.vector.memset(ones_f32, 1.0)
        nc.vector.tensor_copy(Vaug_sb[:, HD::VW], ones_f32)
        nc.vector.memset(ones2_f32, 0.0)
        nc.vector.memset(ones2_f32[0:1, 0:64], 1.0)
        # DVE memset can't start at partition 1; DMA-copy the ones block.
        nc.sync.dma_start(out=ones2_f32[1:2, 64:128],
                          in_=ones2_f32[0:1, 0:64])
        nc.vector.tensor_copy(ones2_sb, ones2_f32)
        ones1_f32 = pA.tile((1, 128), FP32)
        nc.vector.memset(ones1_f32, 1.0)
        nc.vector.tensor_copy(ones1_sb, ones1_f32)

        # Single SP DMA ring, (wq,x) pairs interleaved so the first
        # projection matmul can start after ~1.1MB lands; owT last (phase C
        # only). A second concurrent HWDGE ring (or denser PE packing)
        # removes stalls but burns DEC power budget and the PE gets
        # throttled harder -- net loss, so keep this schedule loose.
        nc.sync.dma_start(out=wqT_sb[:, 0:JG], in_=wqT[0:128, :].bitcast(FP32R))
        for st in range(QT_TILES):
            nc.sync.dma_start(
                out=xT_sb[:, st * 512:(st + 1) * 512],
                in_=xT[0:128, st * 512:(st + 1) * 512].bitcast(FP32R))
        for dc in range(1, DC):
            nc.sync.dma_start(out=wqT_sb[:, dc * JG:(dc + 1) * JG],
                              in_=wqT[dc * 128:(dc + 1) * 128, :].bitcast(FP32R))
            nc.sync.dma_start(out=xT_sb[:, dc * S:(dc + 1) * S],
                              in_=xT[dc * 128:(dc + 1) * 128, :].bitcast(FP32R))
        for jc in range(2):
            nc.sync.dma_start(out=bq_sb[:, jc:jc + 1],
                              in_=bq[jc * 128:(jc + 1) * 128])
        for dc in range(DC):
            nc.sync.dma_start(out=wkT_sb[:, dc * JG:(dc + 1) * JG],
                              in_=wkT[dc * 128:(dc + 1) * 128, :].bitcast(FP32R))
        for jc in range(2):
            nc.sync.dma_start(out=bk_sb[:, jc:jc + 1],
                              in_=bk[jc * 128:(jc + 1) * 128])
        for dc in range(DC):
            nc.sync.dma_start(out=wvT_sb[:, dc * JG:(dc + 1) * JG],
                              in_=wvT[dc * 128:(dc + 1) * 128, :].bitcast(FP32R))
        bv_bcast = bass.AP(tensor=bv.tensor, offset=bv.offset,
                           ap=[[0, 128]] + list(bv.ap))
        nc.gpsimd.dma_start(out=bv_bc, in_=bv_bcast)
        for jc in range(2):
            nc.sync.dma_start(out=owT_sb[:, jc * D:(jc + 1) * D],
                              in_=owT[jc * 128:(jc + 1) * 128, :].bitcast(FP32R))

        # QT / KT: [j-local, s] as 2 chunks of [128, 2048]. dc-outer so the
        # first matmul only needs the first (wq, x) chunk pair off the wire.
        for w_sb, b_sb, dst in ((wqT_sb, bq_sb, QT_sb), (wkT_sb, bk_sb, KT_sb)):
            for jc in range(2):
                pss = [psA.tile((128, 512), FP32, tag=f"mm{st}", bufs=1,
                                name=f"mm{st}") for st in range(QT_TILES)]
                for dc in range(DC):
                    for st in range(QT_TILES):
                        nc.tensor.matmul(
                            pss[st],
                            w_sb[:, dc * JG + jc * 128: dc * JG + (jc + 1) * 128],
                            xT_sb[:, dc * S + st * 512: dc * S + (st + 1) * 512],
                            start=(dc == 0), stop=(dc == DC - 1),
                        )
                for st in range(QT_TILES):
                    nc.vector.tensor_scalar_add(
                        out=dst[:, jc * S + st * 512: jc * S + (st + 1) * 512],
                        in0=pss[st], scalar1=b_sb[:, jc:jc + 1])

        # V: [s, j-local] in 16 chunks, interleaved into Vaug (stride 65)
        for sc in range(KC):
            ps = psA.tile((128, JG), FP32, tag="mmv", bufs=3)
            for dc in range(DC):
                nc.tensor.matmul(
                    ps,
                    xT_sb[:, dc * S + sc * 128: dc * S + (sc + 1) * 128],
                    wvT_sb[:, dc * JG:(dc + 1) * JG],
                    start=(dc == 0), stop=(dc == DC - 1),
                )
            base = sc * 4 * VW
            for a in range(4):
                nc.vector.tensor_add(
                    out=Vaug_sb[:, base + a * VW: base + a * VW + HD],
                    in0=ps[:, a * HD:(a + 1) * HD],
                    in1=bv_bc[:, a * HD:(a + 1) * HD])

    # ---------------- phase B + C: attention, interleaved out-proj ----------
    # PSUM banks: av0(2) + av1(2) + lg0(1) + lg1(1) + op(2, shared with the
    # bcast matmul) = 8. lg bufs=1 is free: the next kc's logits matmul only
    # WAR-waits the previous exp, which drains well within the 4-matmul cycle.
    # Out-projection for qt is emitted one half-block later (after qt+1's
    # pair-0 staging), so its attn_outT deps are long satisfied and the PE
    # never stalls on the normalize chain except at the very end.
    with tc.tile_pool(name="pB", bufs=1) as pB, \
         tc.tile_pool(name="psB", bufs=1, space="PSUM") as psB:

        def emit_outproj_st(st):
            for it in range(2):
                ps = psB.tile((128, 512), FP32, tag="op", bufs=2, name="op")
                for jc in range(2):
                    nc.tensor.matmul(
                        ps,
                        attn_outT_sb[:, jc * S + st * 128:
                                     jc * S + st * 128 + 128],
                        owT_sb[:, jc * D + it * 512: jc * D + (it + 1) * 512],
                        start=(jc == 0), stop=(jc == 1))
                ost = pB.tile((128, 512), FP32, tag="ost", bufs=4, name="ost")
                nc.vector.tensor_copy(ost, ps)
                nc.sync.dma_start(
                    out=out[st * 128:(st + 1) * 128,
                            it * 512:(it + 1) * 512],
                    in_=ost)

        def emit_outproj(qt):
            for st in range(4 * qt, 4 * qt + 4):
                emit_outproj_st(st)

        for qt in range(QT_TILES):
            for pair in range(2):
                h0, h1 = 2 * pair, 2 * pair + 1
                av0 = psB.tile((128, 512), FP32, tag="av0", bufs=2, name="av0")
                av1 = psB.tile((128, 512), FP32, tag="av1", bufs=2, name="av1")
                qcol = pair * S + qt * 512
                for kc in range(KC):
                    lg0 = psB.tile((128, 512), FP32, tag="lg0", bufs=1,
                                   name="lg0")
                    lg1 = psB.tile((128, 512), FP32, tag="lg1", bufs=1,
                                   name="lg1")
                    kcol = pair * S + kc * 128
                    nc.tensor.matmul(
                        lg0,
                        KT_sb[0:64, kcol:kcol + 128],
                        QT_sb[0:64, qcol:qcol + 512],
                        start=True, stop=True, tile_position=(0, 0))
                    nc.tensor.matmul(
                        lg1,
                        KT_sb[64:128, kcol:kcol + 128],
                        QT_sb[64:128, qcol:qcol + 512],
                        start=True, stop=True, tile_position=(64, 0))
                    for h, lg, avp in ((h0, lg0, av0), (h1, lg1, av1)):
                        at = pB.tile((128, 512), FP32R, tag=f"at{h % 2}",
                                     bufs=3, name=f"at{h % 2}")
                        nc.scalar.activation(at, lg, EXP)
                        nc.tensor.matmul(
                            avp[0:VW, :],
                            Vaug_sb[:, kc * 4 * VW + h * VW:
                                    kc * 4 * VW + (h + 1) * VW],
                            at,
                            start=(kc == 0), stop=(kc == KC - 1))
                if qt == QT_TILES - 1 and pair == 1:
                    # last block: skip the DMA partition-scatter hop (two K=1
                    # all-ones broadcasts into the now-free av PSUM banks)
                    # and interleave the first out-proj tile with chunked
                    # recip/muls so the tail chain is as short as possible.
                    d0 = pB.tile((1, 512), FP32R, tag="dstage", bufs=2,
                                 name="d0")
                    nc.scalar.activation(d0, av0[HD:HD + 1, :],
                                         mybir.ActivationFunctionType.Copy)
                    d1 = pB.tile((1, 512), FP32R, tag="drow", bufs=2,
                                 name="d1")
                    nc.vector.tensor_copy(d1, av1[HD:HD + 1, :])
                    bc0 = psB.tile((128, 512), FP32, tag="av0", bufs=2,
                                   name="bc0")
                    nc.tensor.matmul(bc0, ones1_sb, d0, start=True, stop=True)
                    bc1 = psB.tile((128, 512), FP32, tag="av1", bufs=2,
                                   name="bc1")
                    nc.tensor.matmul(bc1, ones1_sb, d1, start=True, stop=True)
                    rcs0 = pB.tile((128, 512), FP32, tag="rcs", bufs=2,
                                   name="rcs0")
                    rcs1 = pB.tile((128, 512), FP32, tag="rcs", bufs=2,
                                   name="rcs1")
                    base = pair * S + qt * 512
                    for lo, hi, sts in ((0, 128, (12,)), (128, 512,
                                                         (13, 14, 15))):
                        nc.vector.reciprocal(rcs0[0:HD, lo:hi],
                                             bc0[0:HD, lo:hi])
                        nc.vector.tensor_mul(
                            out=attn_outT_sb[0:HD, base + lo:base + hi],
                            in0=av0[0:HD, lo:hi], in1=rcs0[0:HD, lo:hi])
                        nc.vector.reciprocal(rcs1[HD:128, lo:hi],
                                             bc1[HD:128, lo:hi])
                        nc.vector.tensor_mul(
                            out=attn_outT_sb[HD:128, base + lo:base + hi],
                            in0=av1[0:HD, lo:hi], in1=rcs1[HD:128, lo:hi])
                        for st in sts:
                            emit_outproj_st(st)
                    continue
                # normalize: pack both heads' denominator rows into one
                # [2,512] tile (ACT copies row 0 while DVE copies row 1),
                # broadcast them with a single K=2 block-ones matmul, then
                # one 128-lane reciprocal feeds both heads' muls.
                # The matmul rhs needs the two rows at partition step 1, but
                # DVE/ACT can't write a partition-1 start. Stage both rows in
                # one partition and let a 4KB SBUF->SBUF DMA scatter them.
                dstage = pB.tile((1, 1024), FP32R, tag="dstage", bufs=2,
                                 name="dstage")
                nc.scalar.activation(dstage[:, 0:512], av0[HD:HD + 1, :],
                                     mybir.ActivationFunctionType.Copy)
                nc.vector.tensor_copy(dstage[:, 512:1024], av1[HD:HD + 1, :])
                drows = pB.tile((2, 512), FP32R, tag="drow", bufs=2,
                                name="drow")
                nc.sync.dma_start(out=drows, in_=dstage)
                bc = psB.tile((128, 512), FP32, tag="op", bufs=2, name="bc")
                nc.tensor.matmul(bc, ones2_sb, drows, start=True, stop=True)
                rcs = pB.tile((128, 512), FP32, tag="rcs", bufs=2, name="rcs")
                nc.vector.reciprocal(rcs, bc)
                base = pair * S + qt * 512
                nc.vector.tensor_mul(out=attn_outT_sb[0:HD, base:base + 512],
                                     in0=av0[0:HD, :], in1=rcs[0:HD, :])
                nc.vector.tensor_mul(out=attn_outT_sb[HD:128, base:base + 512],
                                     in0=av1[0:HD, :], in1=rcs[HD:128, :])
                if pair == 0 and qt > 0:
                    emit_outproj(qt - 1)


_NC = None


def _build_nc():
    global _NC
    if _NC is not None:
        return _NC
    nc = bacc.Bacc("TRN2", target_bir_lowering=False, debug=False,
                   num_devices=NCORES)
    xT = nc.dram_tensor("xT", [D, S], FP32, kind="ExternalInput").ap()
    wqT = nc.dram_tensor("wqT", [D, JG], FP32, kind="ExternalInput").ap()
    wkT = nc.dram_tensor("wkT", [D, JG], FP32, kind="ExternalInput").ap()
    wvT = nc.dram_tensor("wvT", [D, JG], FP32, kind="ExternalInput").ap()
    bq = nc.dram_tensor("bq", [JG], FP32, kind="ExternalInput").ap()
    bk = nc.dram_tensor("bk", [JG], FP32, kind="ExternalInput").ap()
    bv = nc.dram_tensor("bv", [JG], FP32, kind="ExternalInput").ap()
    owT = nc.dram_tensor("owT", [JG, D], FP32, kind="ExternalInput").ap()
    out = nc.dram_tensor("out", [S, D], FP32, kind="ExternalOutput").ap()
    with tile.TileContext(nc) as tc:
        mha_core_kernel(tc, out, xT, wqT, wkT, wvT, bq, bk, bv, owT)
    nc.compile()
    _NC = nc
    return nc


def _in_maps(x, kqv_w, kqv_b, out_w):
    maps = []
    for c in range(NCORES):
        b, g = divmod(c, 4)
        sl = slice(g * JG, (g + 1) * JG)
        maps.append({
            "xT": np.ascontiguousarray(x[b].T),
            "wqT": np.ascontiguousarray(kqv_w[0 * D:1 * D][sl].T),
            "wkT": np.ascontiguousarray(kqv_w[1 * D:2 * D][sl].T),
            "wvT": np.ascontiguousarray(kqv_w[2 * D:3 * D][sl].T),
            "bq": np.ascontiguousarray(kqv_b[0 * D:1 * D][sl]),
            "bk": np.ascontiguousarray(kqv_b[1 * D:2 * D][sl]),
            "bv": np.ascontiguousarray(kqv_b[2 * D:3 * D][sl]),
            "owT": np.ascontiguousarray(out_w[:, sl].T),
        })
    return maps


def run_spmd(x, kqv_w, kqv_b, out_w, out_b, trace=False, tmpdir=None):
    nc = _build_nc()
    res = run_bass_kernel_spmd(nc, _in_maps(x, kqv_w, kqv_b, out_w),
                               list(range(NCORES)), tmpdir=tmpdir, trace=trace)
    parts = [np.asarray(res.results[c]["out"], dtype=np.float32)
             for c in range(NCORES)]
    full = np.stack([
        parts[4 * b] + parts[4 * b + 1] + parts[4 * b + 2] + parts[4 * b + 3]
        + out_b[None, :].astype(np.float32)
        for b in range(B)
    ])
    return full, res


def kernel(**inputs):
    x = np.asarray(inputs["x"], dtype=np.float32)
    kqv_w = np.asarray(inputs["kqv_w"], dtype=np.float32)
    kqv_b = np.asarray(inputs["kqv_b"], dtype=np.float32)
    out_w = np.asarray(inputs["out_w"], dtype=np.float32)
    out_b = np.asarray(inputs["out_b"], dtype=np.float32)
    full, _ = run_spmd(x, kqv_w, kqv_b, out_w, out_b)
    return full



# revision 3
# speedup vs baseline: 1.2322x; 1.2322x over previous
"""MultiHeadAttention (B=2, S=2048, D=1024, H=16) on 8 TRN2 NeuronCores.

Sharding: core c -> batch b = c//4, head-group g = c%4 (4 heads = 256 channels).
Each core computes its 4 heads' attention for its batch plus the partial
out-projection (out_w columns for its channel group); host sums the 4 partials
per batch and adds out_b.

v2 (from the 442us all-fp32r baseline): mixed precision + batched exp.
  - Q/K path stays fp32r (logits precision dominates the error budget);
    V, exp output, AV, attn-out, out-proj and the output DMA are bf16.
    Host-side numpy check: ~3-4e-3 rel err vs the 2e-2 gate.
  - logits for two key-chunks land in one [128,1024] 2-bank PSUM tile so
    each ACTIVATE exps 1024 columns ((N+352)/1.2ns -> 25% fewer ACT ns),
    phase B is Scalar-engine-bound at ~147us.
  - av PSUM single-buffered; the accumulated block is copied to SBUF right
    after the last AV so the bank frees while the (cheap) normalize chain
    runs from SBUF: recip via reciprocal_approx_fast (5x faster than
    reciprocal(), 18 bits is plenty for softmax denominators).
"""

import os
import sys

import numpy as np

for _p in ("/opt/trn_rl_repo",):
    if os.path.isdir(_p) and _p not in sys.path:
        sys.path.insert(0, _p)

from contextlib import ExitStack

import ml_dtypes

import concourse.bass as bass
import concourse.tile as tile
from concourse import bacc, mybir
from concourse._compat import with_exitstack
from concourse.bass_utils import run_bass_kernel_spmd

B, S, D = 2, 2048, 1024
H = 16
HD = 64
NCORES = 8
JG = 256          # channels per core (4 heads)
DC = D // 128     # 8 contraction chunks
QT_TILES = 4      # 4 x 512 query tiles
KC = S // 128     # 16 key chunks
VW = 65           # V columns per head incl. ones column
FP32 = mybir.dt.float32
FP32R = mybir.dt.float32r
BF16 = mybir.dt.bfloat16
EXP = mybir.ActivationFunctionType.Exp


@with_exitstack
def mha_core_kernel(ctx: ExitStack, tc: tile.TileContext,
                    out, xT, wqT, wkT, wvT, bq, bk, bv, owT):
    nc = tc.nc
    # fp32r SBUF tiles hold full fp32 bit patterns; only the PE rounds.
    ctx.enter_context(nc.allow_low_precision("bf16 V/AV/out-proj path"))

    persist = ctx.enter_context(tc.tile_pool(name="persist", bufs=1))
    QT_sb = persist.tile((128, 2 * S), FP32R)
    KT_sb = persist.tile((128, 2 * S), FP32R)
    Vaug_sb = persist.tile((128, KC * 4 * VW), BF16)
    attn_outT_sb = persist.tile((128, 2 * S), BF16)
    owT_sb = persist.tile((128, 2 * D), BF16)
    ones2_sb = persist.tile((2, 128), FP32R)

    # ---------------- phase A: QKV projections ----------------
    with tc.tile_pool(name="pA", bufs=1) as pA, \
         tc.tile_pool(name="psA", bufs=1, space="PSUM") as psA:
        xT_sb = pA.tile((128, DC * S), FP32R)
        wqT_sb = pA.tile((128, DC * JG), FP32R)
        wkT_sb = pA.tile((128, DC * JG), FP32R)
        wvT_sb = pA.tile((128, DC * JG), FP32R)
        bq_sb = pA.tile((128, 2), FP32)
        bk_sb = pA.tile((128, 2), FP32)
        bv_bc = pA.tile((128, JG), FP32)
        ones_f32 = pA.tile((128, 64), FP32)
        ones2_f32 = pA.tile((2, 128), FP32)

        # Stage fp32 ones and DVE-copy (cast) into the bf16 ones columns of
        # Vaug + the fp32r 2-row block-ones used for the denominator bcast.
        nc.vector.memset(ones_f32, 1.0)
        nc.vector.tensor_copy(Vaug_sb[:, HD::VW], ones_f32)
        nc.vector.memset(ones2_f32, 0.0)
        nc.vector.memset(ones2_f32[0:1, 0:64], 1.0)
        # DVE memset can't start at partition 1; DMA-copy the ones block.
        nc.sync.dma_start(out=ones2_f32[1:2, 64:128],
                          in_=ones2_f32[0:1, 0:64])
        nc.vector.tensor_copy(ones2_sb, ones2_f32)

        # Single SP DMA ring, (wq,x) pairs interleaved so the first
        # projection matmul can start after ~1.1MB lands; owT last (phase C
        # only). A second concurrent HWDGE ring (or denser PE packing)
        # removes stalls but burns DEC power budget and the PE gets
        # throttled harder -- net loss, so keep this schedule loose.
        nc.sync.dma_start(out=wqT_sb[:, 0:JG], in_=wqT[0:128, :].bitcast(FP32R))
        for st in range(QT_TILES):
            nc.sync.dma_start(
                out=xT_sb[:, st * 512:(st + 1) * 512],
                in_=xT[0:128, st * 512:(st + 1) * 512].bitcast(FP32R))
        for dc in range(1, DC):
            nc.sync.dma_start(out=wqT_sb[:, dc * JG:(dc + 1) * JG],
                              in_=wqT[dc * 128:(dc + 1) * 128, :].bitcast(FP32R))
            nc.sync.dma_start(out=xT_sb[:, dc * S:(dc + 1) * S],
                              in_=xT[dc * 128:(dc + 1) * 128, :].bitcast(FP32R))
        for jc in range(2):
            nc.sync.dma_start(out=bq_sb[:, jc:jc + 1],
                              in_=bq[jc * 128:(jc + 1) * 128])
        for dc in range(DC):
            nc.sync.dma_start(out=wkT_sb[:, dc * JG:(dc + 1) * JG],
                              in_=wkT[dc * 128:(dc + 1) * 128, :].bitcast(FP32R))
        for jc in range(2):
            nc.sync.dma_start(out=bk_sb[:, jc:jc + 1],
                              in_=bk[jc * 128:(jc + 1) * 128])
        for dc in range(DC):
            nc.sync.dma_start(out=wvT_sb[:, dc * JG:(dc + 1) * JG],
                              in_=wvT[dc * 128:(dc + 1) * 128, :].bitcast(FP32R))
        bv_bcast = bass.AP(tensor=bv.tensor, offset=bv.offset,
                           ap=[[0, 128]] + list(bv.ap))
        nc.gpsimd.dma_start(out=bv_bc, in_=bv_bcast)
        for jc in range(2):
            nc.sync.dma_start(out=owT_sb[:, jc * D:(jc + 1) * D],
                              in_=owT[jc * 128:(jc + 1) * 128, :])

        # QT / KT: [j-local, s] as 2 chunks of [128, 2048]. dc-outer so the
        # first matmul only needs the first (wq, x) chunk pair off the wire.
        for w_sb, b_sb, dst in ((wqT_sb, bq_sb, QT_sb), (wkT_sb, bk_sb, KT_sb)):
            for jc in range(2):
                pss = [psA.tile((128, 512), FP32, tag=f"mm{st}", bufs=1,
                                name=f"mm{st}") for st in range(QT_TILES)]
                for dc in range(DC):
                    for st in range(QT_TILES):
                        nc.tensor.matmul(
                            pss[st],
                            w_sb[:, dc * JG + jc * 128: dc * JG + (jc + 1) * 128],
                            xT_sb[:, dc * S + st * 512: dc * S + (st + 1) * 512],
                            start=(dc == 0), stop=(dc == DC - 1),
                        )
                for st in range(QT_TILES):
                    nc.vector.tensor_scalar_add(
                        out=dst[:, jc * S + st * 512: jc * S + (st + 1) * 512],
                        in0=pss[st], scalar1=b_sb[:, jc:jc + 1])

        # V: [s, j-local] in 16 chunks, interleaved into Vaug (stride 65)
        for sc in range(KC):
            ps = psA.tile((128, JG), FP32, tag="mmv", bufs=3)
            for dc in range(DC):
                nc.tensor.matmul(
                    ps,
                    xT_sb[:, dc * S + sc * 128: dc * S + (sc + 1) * 128],
                    wvT_sb[:, dc * JG:(dc + 1) * JG],
                    start=(dc == 0), stop=(dc == DC - 1),
                )
            base = sc * 4 * VW
            for a in range(4):
                nc.vector.tensor_add(
                    out=Vaug_sb[:, base + a * VW: base + a * VW + HD],
                    in0=ps[:, a * HD:(a + 1) * HD],
                    in1=bv_bc[:, a * HD:(a + 1) * HD])

    # ---------------- phase B + C: attention, interleaved out-proj ----------
    # PSUM banks: av0(1) + av1(1) + lg0(2) + lg1(2) + op(2, shared with the
    # bcast matmul) = 8. The accumulated av block is copied to SBUF (raw0/1)
    # right after its stop-matmul so the single av bank is free well before
    # the next block's start-matmul; the normalize chain runs from SBUF.
    # Out-projection for qt is emitted one half-block later so its
    # attn_outT deps are long satisfied.
    with tc.tile_pool(name="pB", bufs=1) as pB, \
         tc.tile_pool(name="psB", bufs=1, space="PSUM") as psB:

        def emit_outproj_st(st):
            for it in range(2):
                ps = psB.tile((128, 512), FP32, tag="op", bufs=2, name="op")
                for jc in range(2):
                    nc.tensor.matmul(
                        ps,
                        attn_outT_sb[:, jc * S + st * 128:
                                     jc * S + st * 128 + 128],
                        owT_sb[:, jc * D + it * 512: jc * D + (it + 1) * 512],
                        start=(jc == 0), stop=(jc == 1))
                ost = pB.tile((128, 512), BF16, tag="ost", bufs=4, name="ost")
                nc.vector.tensor_copy(ost, ps)
                nc.sync.dma_start(
                    out=out[st * 128:(st + 1) * 128,
                            it * 512:(it + 1) * 512],
                    in_=ost)

        def emit_outproj(qt):
            for st in range(4 * qt, 4 * qt + 4):
                emit_outproj_st(st)

        for qt in range(QT_TILES):
            for pair in range(2):
                h0, h1 = 2 * pair, 2 * pair + 1
                av0 = psB.tile((128, 512), FP32, tag="av0", bufs=1, name="av0")
                av1 = psB.tile((128, 512), FP32, tag="av1", bufs=1, name="av1")
                qcol = pair * S + qt * 512
                for kp in range(KC // 2):
                    kc0, kc1 = 2 * kp, 2 * kp + 1
                    lg0 = psB.tile((128, 1024), FP32, tag="lg0", bufs=1,
                                   name="lg0")
                    lg1 = psB.tile((128, 1024), FP32, tag="lg1", bufs=1,
                                   name="lg1")
                    for i, kc in ((0, kc0), (1, kc1)):
                        kcol = pair * S + kc * 128
                        nc.tensor.matmul(
                            lg0[:, i * 512:(i + 1) * 512],
                            KT_sb[0:64, kcol:kcol + 128],
                            QT_sb[0:64, qcol:qcol + 512],
                            start=True, stop=True, tile_position=(0, 0))
                        nc.tensor.matmul(
                            lg1[:, i * 512:(i + 1) * 512],
                            KT_sb[64:128, kcol:kcol + 128],
                            QT_sb[64:128, qcol:qcol + 512],
                            start=True, stop=True, tile_position=(64, 0))
                    for h, lg, avp in ((h0, lg0, av0), (h1, lg1, av1)):
                        at = pB.tile((128, 1024), BF16, tag=f"at{h % 2}",
                                     bufs=2, name=f"at{h % 2}")
                        nc.scalar.activation(at, lg, EXP)
                        for i, kc in ((0, kc0), (1, kc1)):
                            nc.tensor.matmul(
                                avp[0:VW, :],
                                Vaug_sb[:, kc * 4 * VW + h * VW:
                                        kc * 4 * VW + (h + 1) * VW],
                                at[:, i * 512:(i + 1) * 512],
                                start=(kc == 0), stop=(kc == KC - 1))
                # normalize: move the finished block to SBUF (freeing the av
                # banks; head 1 lands at partitions 64-127 so the muls' SBUF
                # operands share start partitions), pack both denominator
                # rows into one staging row (ACT copies h0 while DVE copies
                # h1), scatter them to [2,512] with one 4KB DMA, broadcast
                # with a single K=2 block-ones matmul, then one fast-approx
                # reciprocal feeds both heads' muls.
                raw0 = pB.tile((HD, 512), FP32, tag="raw0", bufs=2,
                               name="raw0")
                nc.vector.tensor_copy(raw0, av0[0:HD, :])
                raw1 = pB.tile((128, 512), FP32, tag="raw1", bufs=2,
                               name="raw1")
                nc.vector.tensor_copy(raw1[HD:128, :], av1[0:HD, :])
                dstage = pB.tile((1, 1024), FP32R, tag="dstage", bufs=2,
                                 name="dstage")
                nc.scalar.activation(dstage[:, 0:512], av0[HD:HD + 1, :],
                                     mybir.ActivationFunctionType.Copy)
                nc.vector.tensor_copy(dstage[:, 512:1024], av1[HD:HD + 1, :])
                drows = pB.tile((2, 512), FP32R, tag="drow", bufs=2,
                                name="drow")
                nc.sync.dma_start(out=drows, in_=dstage)
                bc = psB.tile((128, 512), FP32, tag="op", bufs=2, name="bc")
                nc.tensor.matmul(bc, ones2_sb, drows, start=True, stop=True)
                rcs = pB.tile((128, 512), FP32, tag="rcs", bufs=2, name="rcs")
                nc.vector.reciprocal_approx_fast(rcs, bc)
                base = pair * S + qt * 512
                nc.vector.tensor_mul(out=attn_outT_sb[0:HD, base:base + 512],
                                     in0=raw0[0:HD, :], in1=rcs[0:HD, :])
                nc.vector.tensor_mul(out=attn_outT_sb[HD:128, base:base + 512],
                                     in0=raw1[HD:128, :], in1=rcs[HD:128, :])
                if pair == 0 and qt > 0:
                    emit_outproj(qt - 1)
        emit_outproj(QT_TILES - 1)


_NC = None


def _build_nc():
    global _NC
    if _NC is not None:
        return _NC
    nc = bacc.Bacc("TRN2", target_bir_lowering=False, debug=False,
                   num_devices=NCORES)
    xT = nc.dram_tensor("xT", [D, S], FP32, kind="ExternalInput").ap()
    wqT = nc.dram_tensor("wqT", [D, JG], FP32, kind="ExternalInput").ap()
    wkT = nc.dram_tensor("wkT", [D, JG], FP32, kind="ExternalInput").ap()
    wvT = nc.dram_tensor("wvT", [D, JG], FP32, kind="ExternalInput").ap()
    bq = nc.dram_tensor("bq", [JG], FP32, kind="ExternalInput").ap()
    bk = nc.dram_tensor("bk", [JG], FP32, kind="ExternalInput").ap()
    bv = nc.dram_tensor("bv", [JG], FP32, kind="ExternalInput").ap()
    owT = nc.dram_tensor("owT", [JG, D], BF16, kind="ExternalInput").ap()
    out = nc.dram_tensor("out", [S, D], BF16, kind="ExternalOutput").ap()
    with tile.TileContext(nc) as tc:
        mha_core_kernel(tc, out, xT, wqT, wkT, wvT, bq, bk, bv, owT)
    nc.compile()
    _NC = nc
    return nc


def _in_maps(x, kqv_w, kqv_b, out_w):
    maps = []
    for c in range(NCORES):
        b, g = divmod(c, 4)
        sl = slice(g * JG, (g + 1) * JG)
        maps.append({
            "xT": np.ascontiguousarray(x[b].T),
            "wqT": np.ascontiguousarray(kqv_w[0 * D:1 * D][sl].T),
            "wkT": np.ascontiguousarray(kqv_w[1 * D:2 * D][sl].T),
            "wvT": np.ascontiguousarray(kqv_w[2 * D:3 * D][sl].T),
            "bq": np.ascontiguousarray(kqv_b[0 * D:1 * D][sl]),
            "bk": np.ascontiguousarray(kqv_b[1 * D:2 * D][sl]),
            "bv": np.ascontiguousarray(kqv_b[2 * D:3 * D][sl]),
            "owT": np.ascontiguousarray(out_w[:, sl].T).astype(
                ml_dtypes.bfloat16),
        })
    return maps


def run_spmd(x, kqv_w, kqv_b, out_w, out_b, trace=False, tmpdir=None):
    nc = _build_nc()
    res = run_bass_kernel_spmd(nc, _in_maps(x, kqv_w, kqv_b, out_w),
                               list(range(NCORES)), tmpdir=tmpdir, trace=trace)
    parts = [np.asarray(res.results[c]["out"]).astype(np.float32)
             for c in range(NCORES)]
    full = np.stack([
        parts[4 * b] + parts[4 * b + 1] + parts[4 * b + 2] + parts[4 * b + 3]
        + out_b[None, :].astype(np.float32)
        for b in range(B)
    ])
    return full, res


def kernel(**inputs):
    x = np.asarray(inputs["x"], dtype=np.float32)
    kqv_w = np.asarray(inputs["kqv_w"], dtype=np.float32)
    kqv_b = np.asarray(inputs["kqv_b"], dtype=np.float32)
    out_w = np.asarray(inputs["out_w"], dtype=np.float32)
    out_b = np.asarray(inputs["out_b"], dtype=np.float32)
    full, _ = run_spmd(x, kqv_w, kqv_b, out_w, out_b)
    return full
